# revision 10
# baseline (speedup 1.0000x reference)
"""BiMamba Trainium2 Bass kernel.

Sharding: data-parallel over batch — 8 NeuronCores, one batch element each,
no collectives. Each core runs both directional Mamba blocks (fwd on x,
bwd on host-flipped x) in channel-major layout (d on partitions, L free).

Per direction (d_model=256, d_inner=512, n_state=16, dt_rank=16, d_conv=4,
L=2048):
  xc = silu(conv1d(W_in_xi @ x) + conv_b)   -- fused into one PE matmul via
       host-built W2[(k,d),e] = in_w[e,d]*conv_w[e,k] over shifted x views
  delta = softplus(W3 @ xc + dt_b),  W3 = dt_w @ xproj_dt  (host-fused)
  B,C   = xproj_bc @ xc               (staged to DRAM, DMA-broadcast per n)
  h_n[t] = exp(A_n*delta[t])*h_n[t-1] + delta[t]*xc[t]*B[n,t]  (DVE scan)
  y = sum_n C_n*h_n (Pool mul + PE identity-accumulate) + xc*D
  out = out_w @ (y * silu(z)),  z = W_z @ x

A_log is structurally log(arange(1..n_state+1)) broadcast over d (the
reference constructs it that way), so A_n is a per-n scalar — asserted at
runtime — allowing exp(A_n*delta) as one activation with a float scale.
"""

import os
from contextlib import ExitStack

import ml_dtypes
import numpy as np

import concourse.bacc as bacc
import concourse.bass as bass
import concourse.mybir as mybir
import concourse.tile as tile

F32 = mybir.dt.float32
BF16 = mybir.dt.bfloat16
AF = mybir.ActivationFunctionType
MUL = mybir.AluOpType.mult
ADD = mybir.AluOpType.add

D_MODEL = 256
N_STATE = 16
D_INNER = 512
DT_RANK = 16
D_CONV = 4
B_SZ, L = 8, 2048
NDT = D_INNER // 128          # 4 d-inner partition tiles
NCH = L // 512                # 4 free-dim chunks of 512
NET = D_MODEL // 128          # 2 d-model partition tiles

SIM_COMPAT = bool(int(os.environ.get("BIMAMBA_SIM", "0")))

bf = ml_dtypes.bfloat16

_CACHE = {}
_LAST = {}


def _build_nc(a_scal):
    """Build the single-core bass module (same NEFF for all 8 cores).
    a_scal: [2][16] python floats — compiled in as activation scales."""
    nc = bacc.Bacc("TRN2", target_bir_lowering=False, debug=False)

    xp_d = nc.dram_tensor("xp", [2, D_MODEL, L + 3], BF16, kind="ExternalInput")
    w2_d = nc.dram_tensor("w2", [2, 8, 128, D_INNER], BF16, kind="ExternalInput")
    bxc_d = nc.dram_tensor("bxc", [2, 1, D_INNER], BF16, kind="ExternalInput")
    wz_d = nc.dram_tensor("wz", [2, 2, 128, D_INNER], BF16, kind="ExternalInput")
    w3_d = nc.dram_tensor("w3", [2, 4, 128, D_INNER], BF16, kind="ExternalInput")
    bdt_d = nc.dram_tensor("bdt", [2, 1, D_INNER], BF16, kind="ExternalInput")
    wbc_d = nc.dram_tensor("wbc", [2, 4, 128, 64], BF16,
                           kind="ExternalInput")
    wo_d = nc.dram_tensor("wo", [2, 4, 128, D_MODEL], BF16, kind="ExternalInput")
    dd_d = nc.dram_tensor("ddiag", [2, 4, 128, 128], BF16, kind="ExternalInput")
    id_d = nc.dram_tensor("ident", [128, 128], BF16, kind="ExternalInput")
    out_d = nc.dram_tensor("out", [2, D_MODEL, L], F32, kind="ExternalOutput")

    with tile.TileContext(nc) as tc, ExitStack() as ctx:
        wpool = ctx.enter_context(tc.tile_pool(name="wpool", bufs=4))
        const = ctx.enter_context(tc.tile_pool(name="const", bufs=1))
        big = ctx.enter_context(tc.tile_pool(name="big", bufs=1))
        scanp = ctx.enter_context(tc.tile_pool(name="scanp", bufs=2))
        yp = ctx.enter_context(tc.tile_pool(name="yp", bufs=4))
        psum = ctx.enter_context(tc.tile_pool(name="psum", bufs=2, space="PSUM"))
        ypsum = ctx.enter_context(tc.tile_pool(name="ypsum", bufs=4, space="PSUM"))
        dram = ctx.enter_context(tc.tile_pool(name="dram", bufs=1, space="DRAM"))

        ones_bf = const.tile([1, 512], BF16)
        nc.vector.memset(ones_bf, 1.0)
        ident_bf = const.tile([128, 128], BF16)
        nc.sync.dma_start(ident_bf, id_d[:, :])

        for di in range(2):
            _build_dir(
                nc, di, a_scal[di],
                xp_d, w2_d, bxc_d, wz_d, w3_d, bdt_d, wbc_d, wo_d, dd_d,
                out_d, wpool, const, big, scanp, yp, psum, ypsum, dram,
                ones_bf, ident_bf,
            )

    nc.compile()
    return nc


def _silu(nc, yp, dst, src_psum):
    if SIM_COMPAT:
        sg = yp.tile(list(dst.shape), F32, name=f"sg_{nc.next_id()}", tag="sg")
        nc.scalar.activation(sg, src_psum, AF.Sigmoid)
        nc.vector.tensor_tensor(dst, sg, src_psum, MUL)
    else:
        nc.scalar.activation(dst, src_psum, AF.Silu)


def _build_dir(nc, di, a_scal, xp_d, w2_d, bxc_d, wz_d, w3_d, bdt_d, wbc_d,
               wo_d, dd_d, out_d, wpool, const, big, scanp, yp, psum,
               ypsum, dram, ones_bf, ident_bf):
    cs = slice  # brevity helper not needed; keep explicit slices below

    # ---------- load x ----------
    x_sb = []
    for t2 in range(NET):
        t = big.tile([128, L + 3], BF16, name=f"x_{di}_{t2}", tag=f"x{t2}", bufs=2)
        nc.sync.dma_start(t, xp_d[di, t2 * 128:(t2 + 1) * 128, :])
        x_sb.append(t)

    ddiag = []
    for dt in range(NDT):
        t = const.tile([128, 128], BF16, name=f"dd_{di}_{dt}")
        nc.sync.dma_start(t, dd_d[di, dt, :, :])
        ddiag.append(t)

    # ---------- in-proj + conv fused -> xc = silu(.) ----------
    xc = [big.tile([128, L], BF16, name=f"xc_{di}_{dt}", tag=f"xc{dt}", bufs=2)
          for dt in range(NDT)]
    bxc_sb = const.tile([1, D_INNER], BF16, name=f"bxc_{di}")
    nc.sync.dma_start(bxc_sb, bxc_d[di, :, :])

    for et in range(NDT):
        w2_et = []
        for ks in range(8):
            w = wpool.tile([128, 128], BF16, name=f"w2_{di}_{et}_{ks}", tag="wk",
                           bufs=10)
            nc.sync.dma_start(w, w2_d[di, ks, :, et * 128:(et + 1) * 128])
            w2_et.append(w)
        for ch in range(NCH):
            pt = psum.tile([128, 512], F32, name=f"pxc_{di}_{et}_{ch}", tag="mm")
            for ks in range(8):
                k, t2 = ks // NET, ks % NET
                rhs = x_sb[t2][:, k + ch * 512: k + ch * 512 + 512]
                nc.tensor.matmul(pt, w2_et[ks], rhs, start=(ks == 0), stop=False)
            nc.tensor.matmul(
                pt, bxc_sb[:, et * 128:(et + 1) * 128], ones_bf[:, 0:512],
                start=False, stop=True)
            _silu(nc, yp, xc[et][:, ch * 512:(ch + 1) * 512], pt)

    # ---------- delta = softplus(W3 @ xc + dt_b)  (as ln(1+exp)) ----------
    delta = [big.tile([128, L], BF16, name=f"de_{di}_{dt}", tag=f"de{dt}",
                      bufs=2)
             for dt in range(NDT)]
    bdt_sb = const.tile([1, D_INNER], BF16, name=f"bdt_{di}")
    nc.sync.dma_start(bdt_sb, bdt_d[di, :, :])
    for mt in range(NDT):
        w3_mt = []
        for ks in range(NDT):
            w = wpool.tile([128, 128], BF16, name=f"w3_{di}_{mt}_{ks}", tag="wk",
                           bufs=10)
            nc.sync.dma_start(w, w3_d[di, ks, :, mt * 128:(mt + 1) * 128])
            w3_mt.append(w)
        for ch in range(NCH):
            pt = psum.tile([128, 512], F32, name=f"pde_{di}_{mt}_{ch}", tag="mm")
            for ks in range(NDT):
                nc.tensor.matmul(pt, w3_mt[ks],
                                 xc[ks][:, ch * 512:(ch + 1) * 512],
                                 start=(ks == 0), stop=False)
            nc.tensor.matmul(
                pt, bdt_sb[:, mt * 128:(mt + 1) * 128], ones_bf[:, 0:512],
                start=False, stop=True)
            dst = delta[mt][:, ch * 512:(ch + 1) * 512]
            tmp = yp.tile([128, 512], F32, name=f"sp_{di}_{mt}_{ch}", tag="sp", bufs=2)
            nc.scalar.activation(tmp, pt, AF.Exp)
            nc.scalar.activation(dst, tmp, AF.Ln, bias=1.0)

    # ---------- B,C rows -> DRAM staging ----------
    bstage = dram.tile([N_STATE, L], BF16, name=f"bst_{di}", tag=f"bst{di}")
    cstage = dram.tile([N_STATE, L], BF16, name=f"cst_{di}", tag=f"cst{di}")
    wbc_sb = []
    for ks in range(NDT):
        w = wpool.tile([128, 64], BF16, name=f"wbc_{di}_{ks}",
                       tag="wbc")
        nc.sync.dma_start(w, wbc_d[di, ks, :, :])
        wbc_sb.append(w)
    for ch in range(NCH):
        pt = psum.tile([64, 512], F32, name=f"pbc_{di}_{ch}", tag="mm")
        for ks in range(NDT):
            nc.tensor.matmul(pt, wbc_sb[ks],
                             xc[ks][:, ch * 512:(ch + 1) * 512],
                             start=(ks == 0), stop=(ks == NDT - 1))
        bb = yp.tile([N_STATE, 512], BF16, name=f"bb_{di}_{ch}", tag="bb")
        cc = yp.tile([N_STATE, 512], BF16, name=f"cc_{di}_{ch}", tag="cc")
        nc.scalar.copy(bb, pt[0:N_STATE, :])
        nc.scalar.copy(cc, pt[32:32 + N_STATE, :])
        nc.sync.dma_start(bstage[:, ch * 512:(ch + 1) * 512], bb)
        nc.sync.dma_start(cstage[:, ch * 512:(ch + 1) * 512], cc)

    # ---------- u = delta * xc (bf16) ----------
    u = [big.tile([128, L], BF16, name=f"u_{di}_{dt}", tag=f"u{dt}")
         for dt in range(NDT)]
    for dt in range(NDT):
        nc.vector.tensor_tensor(u[dt], delta[dt], xc[dt], MUL)

    # ---------- z = silu(Wz @ x), hoisted (groups ACT table use) ----------
    wz_sb = []
    for ks in range(NET):
        w = wpool.tile([128, D_INNER], BF16, name=f"wz_{di}_{ks}", tag="wz")
        nc.sync.dma_start(w, wz_d[di, ks, :, :])
        wz_sb.append(w)
    zs_all = []
    for dt in range(NDT):
        zst = big.tile([128, L], BF16, name=f"zs_{di}_{dt}", tag=f"zs{dt}")
        for ch in range(NCH):
            zp = psum.tile([128, 512], F32, name=f"pz_{di}_{dt}_{ch}", tag="mm")
            for ks in range(NET):
                rhs = x_sb[ks][:, 3 + ch * 512: 3 + ch * 512 + 512]
                nc.tensor.matmul(zp, wz_sb[ks][:, dt * 128:(dt + 1) * 128],
                                 rhs, start=(ks == 0), stop=(ks == NET - 1))
            _silu(nc, yp, zst[:, ch * 512:(ch + 1) * 512], zp)
        zs_all.append(zst)

    # ---------- scan core ----------
    y2 = []
    for dt in range(NDT):
        yps = [ypsum.tile([128, 512], F32, name=f"yps_{di}_{dt}_{c}", tag="y")
               for c in range(NCH)]
        for n in range(N_STATE):
            e1 = (nc.sync, nc.scalar, nc.gpsimd)[n % 3]
            e2 = (nc.scalar, nc.gpsimd, nc.sync)[n % 3]
            bbc = scanp.tile([128, L], BF16, name=f"bbc_{di}_{dt}_{n}",
                             tag="bbc", bufs=4)
            srcb = bstage[n:n + 1, :]
            e1.dma_start(
                bbc, bass.AP(tensor=srcb.tensor, offset=srcb.offset,
                             ap=[[0, 128]] + list(srcb.ap[1:])))
            cbc = scanp.tile([128, L], BF16, name=f"cbc_{di}_{dt}_{n}",
                             tag="cbc", bufs=4)
            srcc = cstage[n:n + 1, :]
            e2.dma_start(
                cbc, bass.AP(tensor=srcc.tensor, offset=srcc.offset,
                             ap=[[0, 128]] + list(srcc.ap[1:])))

            da = scanp.tile([128, L], BF16, name=f"da_{di}_{dt}_{n}", tag="da")
            nc.scalar.activation(da, delta[dt], AF.Exp, scale=float(a_scal[n]))
            dbx = scanp.tile([128, L], BF16, name=f"dbx_{di}_{dt}_{n}",
                             tag="dbx")
            nc.vector.tensor_tensor(dbx, u[dt], bbc, MUL)
            h = scanp.tile([128, L], BF16, name=f"h_{di}_{dt}_{n}", tag="h")
            nc.vector.tensor_tensor_scan(h, da, dbx, 0.0, MUL, ADD)
            hc = scanp.tile([128, L], BF16, name=f"hc_{di}_{dt}_{n}", tag="hc")
            nc.vector.tensor_tensor(hc, h, cbc, MUL)
            for ch in range(NCH):
                nc.tensor.matmul(
                    yps[ch], ident_bf, hc[:, ch * 512:(ch + 1) * 512],
                    start=(n == 0), stop=False)

        # ---- y2 = (y_scan + xc*D) * silu(z) ----
        y2t = big.tile([128, L], BF16, name=f"y2_{di}_{dt}", tag=f"u{dt}")
        for ch in range(NCH):
            nc.tensor.matmul(yps[ch], ddiag[dt],
                             xc[dt][:, ch * 512:(ch + 1) * 512],
                             start=False, stop=True)
            nc.vector.tensor_tensor(
                y2t[:, ch * 512:(ch + 1) * 512], yps[ch],
                zs_all[dt][:, ch * 512:(ch + 1) * 512], MUL)
        y2.append(y2t)

    # ---------- out-proj ----------
    for ot in range(NET):
        wo_sb = []
        for ks in range(NDT):
            w = wpool.tile([128, 128], BF16, name=f"wo_{di}_{ot}_{ks}",
                           tag="wk", bufs=10)
            nc.sync.dma_start(w, wo_d[di, ks, :, ot * 128:(ot + 1) * 128])
            wo_sb.append(w)
        for ch in range(NCH):
            pt = psum.tile([128, 512], F32, name=f"po_{di}_{ot}_{ch}", tag="mm")
            for ks in range(NDT):
                nc.tensor.matmul(pt, wo_sb[ks],
                                 y2[ks][:, ch * 512:(ch + 1) * 512],
                                 start=(ks == 0), stop=(ks == NDT - 1))
            osb = yp.tile([128, 512], F32, name=f"os_{di}_{ot}_{ch}", tag="os", bufs=2)
            nc.scalar.copy(osb, pt)
            nc.sync.dma_start(
                out_d[di, ot * 128:(ot + 1) * 128, ch * 512:(ch + 1) * 512],
                osb)


# ---------------------------------------------------------------------------
# host side
# ---------------------------------------------------------------------------

def _prep_dir(tw):
    in_w = tw["in_w"].astype(np.float64)        # (1024, 256)
    conv_w = tw["conv_w"].astype(np.float64)    # (512, 4)
    conv_b = tw["conv_b"].astype(np.float64)    # (512,)
    xproj = tw["xproj_w"].astype(np.float64)    # (48, 512)
    dt_w = tw["dt_w"].astype(np.float64)        # (512, 16)
    dt_b = tw["dt_b"].astype(np.float64)        # (512,)
    a_log = tw["A_log"].astype(np.float64)      # (512, 16)
    dvec = tw["D"].astype(np.float32)           # (512,)
    out_w = tw["out_w"].astype(np.float64)      # (256, 512)

    win_xi = in_w[:D_INNER]                     # (512, 256)
    win_z = in_w[D_INNER:]                      # (512, 256)

    w2 = np.zeros((8, 128, D_INNER), np.float64)
    for k in range(D_CONV):
        for t2 in range(NET):
            w2[k * NET + t2] = (win_xi[:, t2 * 128:(t2 + 1) * 128].T
                                * conv_w[:, k][None, :])
    bxc = conv_b[None, :]

    wz = np.stack([win_z[:, i * 128:(i + 1) * 128].T for i in range(NET)])

    w3_full = dt_w @ xproj[:DT_RANK]            # (512 di, 512 d)
    w3 = np.stack([w3_full.T[i * 128:(i + 1) * 128] for i in range(NDT)])
    bdt = dt_b[None, :]

    wbc_full = np.zeros((D_INNER, 64), np.float64)
    wbc_full[:, :N_STATE] = xproj[DT_RANK:DT_RANK + N_STATE].T
    wbc_full[:, 32:32 + N_STATE] = xproj[DT_RANK + N_STATE:].T
    wbc = np.stack([wbc_full[i * 128:(i + 1) * 128] for i in range(NDT)])
    wo = np.stack([out_w.T[i * 128:(i + 1) * 128] for i in range(NDT)])

    a_mat = -np.exp(a_log)
    assert np.allclose(a_mat, a_mat[0:1, :], rtol=1e-5, atol=1e-6), \
        "A_log rows differ across d; per-n scalar fast path invalid"
    ddiag = np.stack([np.diag(dvec[i * 128:(i + 1) * 128].astype(np.float64))
                      for i in range(NDT)])
    return dict(w2=w2, bxc=bxc, wz=wz, w3=w3, bdt=bdt, wbc=wbc, wo=wo,
                ddiag=ddiag, a_scal=a_mat[0])


def kernel(**inputs):
    x = np.asarray(inputs["x"], np.float32)     # (8, 256, 2048)

    prep = []
    for tag in ("fwd", "bwd"):
        tw = {k[len(tag) + 1:]: np.asarray(v) for k, v in inputs.items()
              if k.startswith(tag + "_")}
        prep.append(_prep_dir(tw))

    a_scal = [[float(v) for v in p["a_scal"]] for p in prep]
    key = ("nc", str(a_scal))
    if key not in _CACHE:
        _CACHE[key] = _build_nc(a_scal)
    nc = _CACHE[key]

    def st(arrs, dtype):
        return np.ascontiguousarray(
            np.stack([np.asarray(a) for a in arrs]).astype(dtype))

    common = dict(
        w2=st([p["w2"] for p in prep], bf),
        bxc=st([p["bxc"] for p in prep], bf),
        wz=st([p["wz"] for p in prep], bf),
        w3=st([p["w3"] for p in prep], bf),
        bdt=st([p["bdt"] for p in prep], bf),
        wbc=st([p["wbc"] for p in prep], bf),
        wo=st([p["wo"] for p in prep], bf),
        ddiag=st([p["ddiag"] for p in prep], bf),
        ident=np.eye(128, dtype=bf),
    )

    in_maps = []
    for b in range(B_SZ):
        xp = np.zeros((2, D_MODEL, L + 3), bf)
        xp[0, :, 3:] = x[b].astype(bf)
        xp[1, :, 3:] = x[b, :, ::-1].astype(bf)
        in_maps.append(dict(common, xp=xp))

    _LAST["in_maps"] = in_maps

    if SIM_COMPAT:
        from concourse.bass_interp import CoreSim
        nb = int(os.environ.get("BIMAMBA_SIM_NB", "1"))
        res = []
        for b_i in range(nb):
            sim = CoreSim(nc, trace=False)
            for k, v in in_maps[b_i].items():
                sim.tensor(k)[:] = v
            sim.simulate()
            res.append(dict(out=np.array(sim.tensor("out"))))
        while len(res) < B_SZ:
            res.append(res[-1])
    else:
        from concourse.bass_utils import run_bass_kernel_spmd
        r = run_bass_kernel_spmd(nc, in_maps, core_ids=list(range(B_SZ)))
        res = r.results

    out = np.empty((B_SZ, 2 * D_MODEL, L), np.float32)
    for b in range(B_SZ):
        o = res[b]["out"]
        out[b, :D_MODEL] = o[0]
        out[b, D_MODEL:] = o[1][:, ::-1]
    return out


# revision 13
# speedup vs baseline: 1.0722x; 1.0722x over previous
"""BiMamba Trainium2 Bass kernel.

Sharding: data-parallel over batch — 8 NeuronCores, one batch element each,
no collectives. Each core runs both directional Mamba blocks (fwd on x,
bwd on host-flipped x) in channel-major layout (d on partitions, L free).

Per direction (d_model=256, d_inner=512, n_state=16, dt_rank=16, d_conv=4,
L=2048):
  xc = silu(conv1d(W_in_xi @ x) + conv_b)   -- fused into one PE matmul via
       host-built W2[(k,d),e] = in_w[e,d]*conv_w[e,k] over shifted x views
  delta = softplus(W3 @ xc + dt_b),  W3 = dt_w @ xproj_dt  (host-fused)
  B,C   = xproj_bc @ xc               (staged to DRAM, DMA-broadcast per n)
  h_n[t] = exp(A_n*delta[t])*h_n[t-1] + delta[t]*xc[t]*B[n,t]  (DVE scan)
  y = sum_n C_n*h_n (Pool mul + PE identity-accumulate) + xc*D
  out = out_w @ (y * silu(z)),  z = W_z @ x

A_log is structurally log(arange(1..n_state+1)) broadcast over d (the
reference constructs it that way), so A_n is a per-n scalar — asserted at
runtime — allowing exp(A_n*delta) as one activation with a float scale.
"""

import os
from contextlib import ExitStack

import ml_dtypes
import numpy as np

import concourse.bacc as bacc
import concourse.bass as bass
import concourse.mybir as mybir
import concourse.tile as tile

F32 = mybir.dt.float32
BF16 = mybir.dt.bfloat16
AF = mybir.ActivationFunctionType
MUL = mybir.AluOpType.mult
ADD = mybir.AluOpType.add

D_MODEL = 256
N_STATE = 16
D_INNER = 512
DT_RANK = 16
D_CONV = 4
B_SZ, L = 8, 2048
NDT = D_INNER // 128          # 4 d-inner partition tiles
NCH = L // 512                # 4 free-dim chunks of 512
NET = D_MODEL // 128          # 2 d-model partition tiles

SIM_COMPAT = bool(int(os.environ.get("BIMAMBA_SIM", "0")))

bf = ml_dtypes.bfloat16

_CACHE = {}
_LAST = {}


def _build_nc(a_scal):
    """Build the single-core bass module (same NEFF for all 8 cores).
    a_scal: [2][16] python floats — compiled in as activation scales."""
    nc = bacc.Bacc("TRN2", target_bir_lowering=False, debug=False)

    xp_d = nc.dram_tensor("xp", [2, D_MODEL, L + 3], BF16, kind="ExternalInput")
    w2_d = nc.dram_tensor("w2", [2, 8, 128, D_INNER], BF16, kind="ExternalInput")
    bxc_d = nc.dram_tensor("bxc", [2, 1, D_INNER], BF16, kind="ExternalInput")
    wz_d = nc.dram_tensor("wz", [2, 2, 128, D_INNER], BF16, kind="ExternalInput")
    w3_d = nc.dram_tensor("w3", [2, 4, 128, D_INNER], BF16, kind="ExternalInput")
    bdt_d = nc.dram_tensor("bdt", [2, 1, D_INNER], BF16, kind="ExternalInput")
    wbc_d = nc.dram_tensor("wbc", [2, 4, 128, 64], BF16,
                           kind="ExternalInput")
    wo_d = nc.dram_tensor("wo", [2, 4, 128, D_MODEL], BF16, kind="ExternalInput")
    dd_d = nc.dram_tensor("ddiag", [2, 4, 128, 128], BF16, kind="ExternalInput")
    id_d = nc.dram_tensor("ident", [128, 128], BF16, kind="ExternalInput")
    out_d = nc.dram_tensor("out", [2, D_MODEL, L], F32, kind="ExternalOutput")

    with tile.TileContext(nc) as tc, ExitStack() as ctx:
        wpool = ctx.enter_context(tc.tile_pool(name="wpool", bufs=4))
        const = ctx.enter_context(tc.tile_pool(name="const", bufs=1))
        big = ctx.enter_context(tc.tile_pool(name="big", bufs=1))
        scanp = ctx.enter_context(tc.tile_pool(name="scanp", bufs=2))
        yp = ctx.enter_context(tc.tile_pool(name="yp", bufs=4))
        psum = ctx.enter_context(tc.tile_pool(name="psum", bufs=2, space="PSUM"))
        ypsum = ctx.enter_context(tc.tile_pool(name="ypsum", bufs=4, space="PSUM"))
        dram = ctx.enter_context(tc.tile_pool(name="dram", bufs=1, space="DRAM"))

        ones_bf = const.tile([1, 512], BF16)
        nc.vector.memset(ones_bf, 1.0)
        ident_bf = const.tile([128, 128], BF16)
        nc.sync.dma_start(ident_bf, id_d[:, :])

        pools = dict(wpool=wpool, const=const, big=big, scanp=scanp, yp=yp,
                     psum=psum, ypsum=ypsum, dram=dram)
        tens = dict(xp_d=xp_d, w2_d=w2_d, bxc_d=bxc_d, wz_d=wz_d, w3_d=w3_d,
                    bdt_d=bdt_d, wbc_d=wbc_d, wo_d=wo_d, dd_d=dd_d,
                    out_d=out_d, ones_bf=ones_bf, ident_bf=ident_bf)
        st0 = _phase_a(nc, 0, a_scal[0], pools, tens)
        _phase_scan(nc, 0, st0, pools, tens, dts=[0])
        st1 = _phase_a(nc, 1, a_scal[1], pools, tens)
        _phase_scan(nc, 0, st0, pools, tens, dts=[1, 2, 3])
        _phase_tail(nc, 0, st0, pools, tens)
        _phase_scan(nc, 1, st1, pools, tens, dts=[0, 1, 2, 3])
        _phase_tail(nc, 1, st1, pools, tens)

    nc.compile()
    return nc


def _silu(nc, yp, dst, src_psum):
    if SIM_COMPAT:
        sg = yp.tile(list(dst.shape), F32, name=f"sg_{nc.next_id()}", tag="sg")
        nc.scalar.activation(sg, src_psum, AF.Sigmoid)
        nc.vector.tensor_tensor(dst, sg, src_psum, MUL)
    else:
        nc.scalar.activation(dst, src_psum, AF.Silu)


def _phase_a(nc, di, a_scal, pools, tens):
    """Projections: x load, in-proj+conv->xc, delta, B/C staging, u, z."""
    wpool, const, big, yp = (pools[k] for k in ("wpool", "const", "big", "yp"))
    psum, dram = pools["psum"], pools["dram"]
    xp_d, w2_d, bxc_d = tens["xp_d"], tens["w2_d"], tens["bxc_d"]
    wz_d, w3_d, bdt_d, wbc_d = tens["wz_d"], tens["w3_d"], tens["bdt_d"], tens["wbc_d"]
    dd_d, ones_bf = tens["dd_d"], tens["ones_bf"]

    x_sb = []
    for t2 in range(NET):
        t = big.tile([128, L + 3], BF16, name=f"x_{di}_{t2}", tag=f"x{t2}", bufs=2)
        nc.sync.dma_start(t, xp_d[di, t2 * 128:(t2 + 1) * 128, :])
        x_sb.append(t)

    ddiag = []
    for dt in range(NDT):
        t = const.tile([128, 128], BF16, name=f"dd_{di}_{dt}")
        nc.sync.dma_start(t, dd_d[di, dt, :, :])
        ddiag.append(t)

    # in-proj + conv fused -> xc = silu(.)
    xc = [big.tile([128, L], BF16, name=f"xc_{di}_{dt}", tag=f"xc{dt}", bufs=2)
          for dt in range(NDT)]
    bxc_sb = const.tile([1, D_INNER], BF16, name=f"bxc_{di}")
    nc.sync.dma_start(bxc_sb, bxc_d[di, :, :])

    for et in range(NDT):
        w2_et = []
        for ks in range(8):
            w = wpool.tile([128, 128], BF16, name=f"w2_{di}_{et}_{ks}", tag="wk",
                           bufs=16)
            nc.sync.dma_start(w, w2_d[di, ks, :, et * 128:(et + 1) * 128])
            w2_et.append(w)
        for ch in range(NCH):
            pt = psum.tile([128, 512], F32, name=f"pxc_{di}_{et}_{ch}", tag="mm")
            for ks in range(8):
                k, t2 = ks // NET, ks % NET
                rhs = x_sb[t2][:, k + ch * 512: k + ch * 512 + 512]
                nc.tensor.matmul(pt, w2_et[ks], rhs, start=(ks == 0), stop=False)
            nc.tensor.matmul(
                pt, bxc_sb[:, et * 128:(et + 1) * 128], ones_bf[:, 0:512],
                start=False, stop=True)
            _silu(nc, yp, xc[et][:, ch * 512:(ch + 1) * 512], pt)

    # delta = softplus(W3 @ xc + dt_b) as ln(1+exp)
    delta = [big.tile([128, L], BF16, name=f"de_{di}_{dt}", tag=f"de{dt}",
                      bufs=2)
             for dt in range(NDT)]
    bdt_sb = const.tile([1, D_INNER], BF16, name=f"bdt_{di}")
    nc.sync.dma_start(bdt_sb, bdt_d[di, :, :])
    for mt in range(NDT):
        w3_mt = []
        for ks in range(NDT):
            w = wpool.tile([128, 128], BF16, name=f"w3_{di}_{mt}_{ks}", tag="wk",
                           bufs=16)
            nc.sync.dma_start(w, w3_d[di, ks, :, mt * 128:(mt + 1) * 128])
            w3_mt.append(w)
        for ch in range(NCH):
            pt = psum.tile([128, 512], F32, name=f"pde_{di}_{mt}_{ch}", tag="mm")
            for ks in range(NDT):
                nc.tensor.matmul(pt, w3_mt[ks],
                                 xc[ks][:, ch * 512:(ch + 1) * 512],
                                 start=(ks == 0), stop=False)
            nc.tensor.matmul(
                pt, bdt_sb[:, mt * 128:(mt + 1) * 128], ones_bf[:, 0:512],
                start=False, stop=True)
            dst = delta[mt][:, ch * 512:(ch + 1) * 512]
            tmp = yp.tile([128, 512], F32, name=f"sp_{di}_{mt}_{ch}", tag="sp",
                          bufs=2)
            nc.scalar.activation(tmp, pt, AF.Exp)
            nc.scalar.activation(dst, tmp, AF.Ln, bias=1.0)

    # B,C rows -> DRAM staging (bf16)
    bstage = dram.tile([N_STATE, L], BF16, name=f"bst_{di}", tag=f"bst{di}")
    cstage = dram.tile([N_STATE, L], BF16, name=f"cst_{di}", tag=f"cst{di}")
    wbc_sb = []
    for ks in range(NDT):
        w = wpool.tile([128, 64], BF16, name=f"wbc_{di}_{ks}", tag="wbc")
        nc.sync.dma_start(w, wbc_d[di, ks, :, :])
        wbc_sb.append(w)
    for ch in range(NCH):
        pt = psum.tile([64, 512], F32, name=f"pbc_{di}_{ch}", tag="mm")
        for ks in range(NDT):
            nc.tensor.matmul(pt, wbc_sb[ks],
                             xc[ks][:, ch * 512:(ch + 1) * 512],
                             start=(ks == 0), stop=(ks == NDT - 1))
        bb = yp.tile([N_STATE, 512], BF16, name=f"bb_{di}_{ch}", tag="bb", bufs=2)
        cc = yp.tile([N_STATE, 512], BF16, name=f"cc_{di}_{ch}", tag="cc", bufs=2)
        nc.scalar.copy(bb, pt[0:N_STATE, :])
        nc.scalar.copy(cc, pt[32:32 + N_STATE, :])
        nc.sync.dma_start(bstage[:, ch * 512:(ch + 1) * 512], bb)
        nc.sync.dma_start(cstage[:, ch * 512:(ch + 1) * 512], cc)

    # u = delta * xc (bf16)
    u = [big.tile([128, L], BF16, name=f"u_{di}_{dt}", tag=f"u{dt}")
         for dt in range(NDT)]
    for dt in range(NDT):
        nc.vector.tensor_tensor(u[dt], delta[dt], xc[dt], MUL)

    # z = silu(Wz @ x)
    wz_sb = []
    for ks in range(NET):
        w = wpool.tile([128, D_INNER], BF16, name=f"wz_{di}_{ks}", tag="wz")
        nc.sync.dma_start(w, wz_d[di, ks, :, :])
        wz_sb.append(w)
    zs_all = []
    for dt in range(NDT):
        zst = big.tile([128, L], BF16, name=f"zs_{di}_{dt}", tag=f"zs{dt}")
        for ch in range(NCH):
            zp = psum.tile([128, 512], F32, name=f"pz_{di}_{dt}_{ch}", tag="mm")
            for ks in range(NET):
                rhs = x_sb[ks][:, 3 + ch * 512: 3 + ch * 512 + 512]
                nc.tensor.matmul(zp, wz_sb[ks][:, dt * 128:(dt + 1) * 128],
                                 rhs, start=(ks == 0), stop=(ks == NET - 1))
            _silu(nc, yp, zst[:, ch * 512:(ch + 1) * 512], zp)
        zs_all.append(zst)

    return dict(a_scal=a_scal, x_sb=x_sb, ddiag=ddiag, xc=xc, delta=delta,
                bstage=bstage, cstage=cstage, u=u, zs_all=zs_all, y2={})


def _phase_scan(nc, di, st, pools, tens, dts):
    scanp, big, ypsum = pools["scanp"], pools["big"], pools["ypsum"]
    ident_bf = tens["ident_bf"]
    a_scal = st["a_scal"]
    for dt in dts:
        yps = [ypsum.tile([128, 512], F32, name=f"yps_{di}_{dt}_{c}", tag="y")
               for c in range(NCH)]
        for n in range(N_STATE):
            e1 = (nc.sync, nc.scalar, nc.gpsimd)[n % 3]
            e2 = (nc.scalar, nc.gpsimd, nc.sync)[n % 3]
            bbc = scanp.tile([128, L], BF16, name=f"bbc_{di}_{dt}_{n}",
                             tag="bbc", bufs=4)
            srcb = st["bstage"][n:n + 1, :]
            e1.dma_start(
                bbc, bass.AP(tensor=srcb.tensor, offset=srcb.offset,
                             ap=[[0, 128]] + list(srcb.ap[1:])))
            cbc = scanp.tile([128, L], BF16, name=f"cbc_{di}_{dt}_{n}",
                             tag="cbc", bufs=4)
            srcc = st["cstage"][n:n + 1, :]
            e2.dma_start(
                cbc, bass.AP(tensor=srcc.tensor, offset=srcc.offset,
                             ap=[[0, 128]] + list(srcc.ap[1:])))

            da = scanp.tile([128, L], BF16, name=f"da_{di}_{dt}_{n}", tag="da")
            nc.scalar.activation(da, st["delta"][dt], AF.Exp,
                                 scale=float(a_scal[n]))
            dbx = scanp.tile([128, L], BF16, name=f"dbx_{di}_{dt}_{n}",
                             tag="dbx")
            nc.vector.tensor_tensor(dbx, st["u"][dt], bbc, MUL)
            h = scanp.tile([128, L], BF16, name=f"h_{di}_{dt}_{n}", tag="h")
            nc.vector.tensor_tensor_scan(h, da, dbx, 0.0, MUL, ADD)
            hc = scanp.tile([128, L], BF16, name=f"hc_{di}_{dt}_{n}", tag="hc")
            nc.vector.tensor_tensor(hc, h, cbc, MUL)
            for ch in range(NCH):
                nc.tensor.matmul(
                    yps[ch], ident_bf, hc[:, ch * 512:(ch + 1) * 512],
                    start=(n == 0), stop=False)

        # y2 = (y_scan + xc*D) * silu(z)
        y2t = big.tile([128, L], BF16, name=f"y2_{di}_{dt}", tag=f"de{dt}",
                       bufs=2)
        for ch in range(NCH):
            nc.tensor.matmul(yps[ch], st["ddiag"][dt],
                             st["xc"][dt][:, ch * 512:(ch + 1) * 512],
                             start=False, stop=True)
            nc.vector.tensor_tensor(
                y2t[:, ch * 512:(ch + 1) * 512], yps[ch],
                st["zs_all"][dt][:, ch * 512:(ch + 1) * 512], MUL)
        st["y2"][dt] = y2t


def _phase_tail(nc, di, st, pools, tens):
    wpool, yp, psum = pools["wpool"], pools["yp"], pools["psum"]
    wo_d, out_d = tens["wo_d"], tens["out_d"]
    for ot in range(NET):
        wo_sb = []
        for ks in range(NDT):
            w = wpool.tile([128, 128], BF16, name=f"wo_{di}_{ot}_{ks}",
                           tag="wk", bufs=16)
            nc.sync.dma_start(w, wo_d[di, ks, :, ot * 128:(ot + 1) * 128])
            wo_sb.append(w)
        for ch in range(NCH):
            pt = psum.tile([128, 512], F32, name=f"po_{di}_{ot}_{ch}", tag="mm")
            for ks in range(NDT):
                nc.tensor.matmul(pt, wo_sb[ks],
                                 st["y2"][ks][:, ch * 512:(ch + 1) * 512],
                                 start=(ks == 0), stop=(ks == NDT - 1))
            osb = yp.tile([128, 512], F32, name=f"os_{di}_{ot}_{ch}", tag="os",
                          bufs=2)
            nc.scalar.copy(osb, pt)
            nc.sync.dma_start(
                out_d[di, ot * 128:(ot + 1) * 128, ch * 512:(ch + 1) * 512],
                osb)


# ---------------------------------------------------------------------------
# host side
# ---------------------------------------------------------------------------

def _prep_dir(tw):
    in_w = tw["in_w"].astype(np.float64)        # (1024, 256)
    conv_w = tw["conv_w"].astype(np.float64)    # (512, 4)
    conv_b = tw["conv_b"].astype(np.float64)    # (512,)
    xproj = tw["xproj_w"].astype(np.float64)    # (48, 512)
    dt_w = tw["dt_w"].astype(np.float64)        # (512, 16)
    dt_b = tw["dt_b"].astype(np.float64)        # (512,)
    a_log = tw["A_log"].astype(np.float64)      # (512, 16)
    dvec = tw["D"].astype(np.float32)           # (512,)
    out_w = tw["out_w"].astype(np.float64)      # (256, 512)

    win_xi = in_w[:D_INNER]                     # (512, 256)
    win_z = in_w[D_INNER:]                      # (512, 256)

    w2 = np.zeros((8, 128, D_INNER), np.float64)
    for k in range(D_CONV):
        for t2 in range(NET):
            w2[k * NET + t2] = (win_xi[:, t2 * 128:(t2 + 1) * 128].T
                                * conv_w[:, k][None, :])
    bxc = conv_b[None, :]

    wz = np.stack([win_z[:, i * 128:(i + 1) * 128].T for i in range(NET)])

    w3_full = dt_w @ xproj[:DT_RANK]            # (512 di, 512 d)
    w3 = np.stack([w3_full.T[i * 128:(i + 1) * 128] for i in range(NDT)])
    bdt = dt_b[None, :]

    wbc_full = np.zeros((D_INNER, 64), np.float64)
    wbc_full[:, :N_STATE] = xproj[DT_RANK:DT_RANK + N_STATE].T
    wbc_full[:, 32:32 + N_STATE] = xproj[DT_RANK + N_STATE:].T
    wbc = np.stack([wbc_full[i * 128:(i + 1) * 128] for i in range(NDT)])
    wo = np.stack([out_w.T[i * 128:(i + 1) * 128] for i in range(NDT)])

    a_mat = -np.exp(a_log)
    assert np.allclose(a_mat, a_mat[0:1, :], rtol=1e-5, atol=1e-6), \
        "A_log rows differ across d; per-n scalar fast path invalid"
    ddiag = np.stack([np.diag(dvec[i * 128:(i + 1) * 128].astype(np.float64))
                      for i in range(NDT)])
    return dict(w2=w2, bxc=bxc, wz=wz, w3=w3, bdt=bdt, wbc=wbc, wo=wo,
                ddiag=ddiag, a_scal=a_mat[0])


def kernel(**inputs):
    x = np.asarray(inputs["x"], np.float32)     # (8, 256, 2048)

    prep = []
    for tag in ("fwd", "bwd"):
        tw = {k[len(tag) + 1:]: np.asarray(v) for k, v in inputs.items()
              if k.startswith(tag + "_")}
        prep.append(_prep_dir(tw))

    a_scal = [[float(v) for v in p["a_scal"]] for p in prep]
    key = ("nc", str(a_scal))
    if key not in _CACHE:
        _CACHE[key] = _build_nc(a_scal)
    nc = _CACHE[key]

    def st(arrs, dtype):
        return np.ascontiguousarray(
            np.stack([np.asarray(a) for a in arrs]).astype(dtype))

    common = dict(
        w2=st([p["w2"] for p in prep], bf),
        bxc=st([p["bxc"] for p in prep], bf),
        wz=st([p["wz"] for p in prep], bf),
        w3=st([p["w3"] for p in prep], bf),
        bdt=st([p["bdt"] for p in prep], bf),
        wbc=st([p["wbc"] for p in prep], bf),
        wo=st([p["wo"] for p in prep], bf),
        ddiag=st([p["ddiag"] for p in prep], bf),
        ident=np.eye(128, dtype=bf),
    )

    in_maps = []
    for b in range(B_SZ):
        xp = np.zeros((2, D_MODEL, L + 3), bf)
        xp[0, :, 3:] = x[b].astype(bf)
        xp[1, :, 3:] = x[b, :, ::-1].astype(bf)
        in_maps.append(dict(common, xp=xp))

    _LAST["in_maps"] = in_maps

    if SIM_COMPAT:
        from concourse.bass_interp import CoreSim
        nb = int(os.environ.get("BIMAMBA_SIM_NB", "1"))
        res = []
        for b_i in range(nb):
            sim = CoreSim(nc, trace=False)
            for k, v in in_maps[b_i].items():
                sim.tensor(k)[:] = v
            sim.simulate()
            res.append(dict(out=np.array(sim.tensor("out"))))
        while len(res) < B_SZ:
            res.append(res[-1])
    else:
        from concourse.bass_utils import run_bass_kernel_spmd
        r = run_bass_kernel_spmd(nc, in_maps, core_ids=list(range(B_SZ)))
        res = r.results

    out = np.empty((B_SZ, 2 * D_MODEL, L), np.float32)
    for b in range(B_SZ):
        o = res[b]["out"]
        out[b, :D_MODEL] = o[0]
        out[b, D_MODEL:] = o[1][:, ::-1]
    return out


# revision 15
# speedup vs baseline: 1.0792x; 1.0065x over previous
"""BiMamba Trainium2 Bass kernel.

Sharding: data-parallel over batch — 8 NeuronCores, one batch element each,
no collectives. Each core runs both directional Mamba blocks (fwd on x,
bwd on host-flipped x) in channel-major layout (d on partitions, L free).

Per direction (d_model=256, d_inner=512, n_state=16, dt_rank=16, d_conv=4,
L=2048):
  xc = silu(conv1d(W_in_xi @ x) + conv_b)   -- fused into one PE matmul via
       host-built W2[(k,d),e] = in_w[e,d]*conv_w[e,k] over shifted x views
  delta = softplus(W3 @ xc + dt_b),  W3 = dt_w @ xproj_dt  (host-fused)
  B,C   = xproj_bc @ xc               (staged to DRAM, DMA-broadcast per n)
  h_n[t] = exp(A_n*delta[t])*h_n[t-1] + delta[t]*xc[t]*B[n,t]  (DVE scan)
  y = sum_n C_n*h_n (Pool mul + PE identity-accumulate) + xc*D
  out = out_w @ (y * silu(z)),  z = W_z @ x

A_log is structurally log(arange(1..n_state+1)) broadcast over d (the
reference constructs it that way), so A_n is a per-n scalar — asserted at
runtime — allowing exp(A_n*delta) as one activation with a float scale.
"""

import os
from contextlib import ExitStack

import ml_dtypes
import numpy as np

import concourse.bacc as bacc
import concourse.bass as bass
import concourse.mybir as mybir
import concourse.tile as tile

F32 = mybir.dt.float32
BF16 = mybir.dt.bfloat16
AF = mybir.ActivationFunctionType
MUL = mybir.AluOpType.mult
ADD = mybir.AluOpType.add

D_MODEL = 256
N_STATE = 16
D_INNER = 512
DT_RANK = 16
D_CONV = 4
B_SZ, L = 8, 2048
NDT = D_INNER // 128          # 4 d-inner partition tiles
NCH = L // 512                # 4 free-dim chunks of 512
NET = D_MODEL // 128          # 2 d-model partition tiles

SIM_COMPAT = bool(int(os.environ.get("BIMAMBA_SIM", "0")))

bf = ml_dtypes.bfloat16

_CACHE = {}
_LAST = {}


def _build_nc(a_scal):
    """Build the single-core bass module (same NEFF for all 8 cores).
    a_scal: [2][16] python floats — compiled in as activation scales."""
    nc = bacc.Bacc("TRN2", target_bir_lowering=False, debug=False)

    xp_d = nc.dram_tensor("xp", [2, D_MODEL, L + 3], BF16, kind="ExternalInput")
    w2_d = nc.dram_tensor("w2", [2, 8, 128, D_INNER], BF16, kind="ExternalInput")
    bxc_d = nc.dram_tensor("bxc", [2, 1, D_INNER], BF16, kind="ExternalInput")
    wz_d = nc.dram_tensor("wz", [2, 2, 128, D_INNER], BF16, kind="ExternalInput")
    w3_d = nc.dram_tensor("w3", [2, 4, 128, D_INNER], BF16, kind="ExternalInput")
    bdt_d = nc.dram_tensor("bdt", [2, 1, D_INNER], BF16, kind="ExternalInput")
    wbc_d = nc.dram_tensor("wbc", [2, 4, 128, 64], BF16,
                           kind="ExternalInput")
    wo_d = nc.dram_tensor("wo", [2, 4, 128, D_MODEL], BF16, kind="ExternalInput")
    dd_d = nc.dram_tensor("ddiag", [2, 4, 128, 128], BF16, kind="ExternalInput")
    id_d = nc.dram_tensor("ident", [128, 128], BF16, kind="ExternalInput")
    out_d = nc.dram_tensor("out", [2, D_MODEL, L], F32, kind="ExternalOutput")

    with tile.TileContext(nc) as tc, ExitStack() as ctx:
        wpool = ctx.enter_context(tc.tile_pool(name="wpool", bufs=4))
        const = ctx.enter_context(tc.tile_pool(name="const", bufs=1))
        big = ctx.enter_context(tc.tile_pool(name="big", bufs=1))
        scanp = ctx.enter_context(tc.tile_pool(name="scanp", bufs=2))
        yp = ctx.enter_context(tc.tile_pool(name="yp", bufs=4))
        psum = ctx.enter_context(tc.tile_pool(name="psum", bufs=2, space="PSUM"))
        ypsum = ctx.enter_context(tc.tile_pool(name="ypsum", bufs=4, space="PSUM"))
        dram = ctx.enter_context(tc.tile_pool(name="dram", bufs=1, space="DRAM"))

        ones_bf = const.tile([1, 512], BF16)
        nc.vector.memset(ones_bf, 1.0)
        ident_bf = const.tile([128, 128], BF16)
        nc.sync.dma_start(ident_bf, id_d[:, :])

        pools = dict(wpool=wpool, const=const, big=big, scanp=scanp, yp=yp,
                     psum=psum, ypsum=ypsum, dram=dram)
        tens = dict(xp_d=xp_d, w2_d=w2_d, bxc_d=bxc_d, wz_d=wz_d, w3_d=w3_d,
                    bdt_d=bdt_d, wbc_d=wbc_d, wo_d=wo_d, dd_d=dd_d,
                    out_d=out_d, ones_bf=ones_bf, ident_bf=ident_bf)
        st0 = _phase_a(nc, 0, a_scal[0], pools, tens)
        _phase_scan(nc, 0, st0, pools, tens, dts=[0])
        st1 = _phase_a(nc, 1, a_scal[1], pools, tens)
        _phase_scan(nc, 0, st0, pools, tens, dts=[1, 2, 3])
        _phase_tail(nc, 0, st0, pools, tens)
        _phase_scan(nc, 1, st1, pools, tens, dts=[0, 1, 2, 3])
        _phase_tail(nc, 1, st1, pools, tens)

    nc.compile()
    return nc


def _silu(nc, yp, dst, src_psum):
    if SIM_COMPAT:
        sg = yp.tile(list(dst.shape), F32, name=f"sg_{nc.next_id()}", tag="sg",
                     bufs=2)
        nc.scalar.activation(sg, src_psum, AF.Sigmoid)
        nc.vector.tensor_tensor(dst, sg, src_psum, MUL)
    else:
        nc.scalar.activation(dst, src_psum, AF.Silu)


def _phase_a(nc, di, a_scal, pools, tens):
    """Projections: x load, in-proj+conv->xc, delta, B/C staging, u, z."""
    wpool, const, big, yp = (pools[k] for k in ("wpool", "const", "big", "yp"))
    psum, dram = pools["psum"], pools["dram"]
    xp_d, w2_d, bxc_d = tens["xp_d"], tens["w2_d"], tens["bxc_d"]
    wz_d, w3_d, bdt_d, wbc_d = tens["wz_d"], tens["w3_d"], tens["bdt_d"], tens["wbc_d"]
    dd_d, ones_bf = tens["dd_d"], tens["ones_bf"]

    x_sb = []
    for t2 in range(NET):
        t = big.tile([128, L + 3], BF16, name=f"x_{di}_{t2}", tag=f"x{t2}", bufs=2)
        nc.sync.dma_start(t, xp_d[di, t2 * 128:(t2 + 1) * 128, :])
        x_sb.append(t)

    ddiag = []
    for dt in range(NDT):
        t = const.tile([128, 128], BF16, name=f"dd_{di}_{dt}")
        nc.sync.dma_start(t, dd_d[di, dt, :, :])
        ddiag.append(t)

    # in-proj + conv fused -> xc = silu(.)
    xc = [big.tile([128, L], BF16, name=f"xc_{di}_{dt}", tag=f"xc{dt}", bufs=2)
          for dt in range(NDT)]
    bxc_sb = const.tile([1, D_INNER], BF16, name=f"bxc_{di}")
    nc.sync.dma_start(bxc_sb, bxc_d[di, :, :])

    for et in range(NDT):
        w2_et = []
        for ks in range(8):
            w = wpool.tile([128, 128], BF16, name=f"w2_{di}_{et}_{ks}", tag="wk",
                           bufs=16)
            nc.sync.dma_start(w, w2_d[di, ks, :, et * 128:(et + 1) * 128])
            w2_et.append(w)
        for ch in range(NCH):
            pt = psum.tile([128, 512], F32, name=f"pxc_{di}_{et}_{ch}", tag="mm")
            for ks in range(8):
                k, t2 = ks // NET, ks % NET
                rhs = x_sb[t2][:, k + ch * 512: k + ch * 512 + 512]
                nc.tensor.matmul(pt, w2_et[ks], rhs, start=(ks == 0), stop=False)
            nc.tensor.matmul(
                pt, bxc_sb[:, et * 128:(et + 1) * 128], ones_bf[:, 0:512],
                start=False, stop=True)
            _silu(nc, yp, xc[et][:, ch * 512:(ch + 1) * 512], pt)

    # B,C rows -> DRAM staging (bf16)
    bstage = dram.tile([N_STATE, L], BF16, name=f"bst_{di}", tag=f"bst{di}")
    cstage = dram.tile([N_STATE, L], BF16, name=f"cst_{di}", tag=f"cst{di}")
    wbc_sb = []
    for ks in range(NDT):
        w = wpool.tile([128, 64], BF16, name=f"wbc_{di}_{ks}", tag="wbc")
        nc.sync.dma_start(w, wbc_d[di, ks, :, :])
        wbc_sb.append(w)
    for ch in range(NCH):
        pt = psum.tile([64, 512], F32, name=f"pbc_{di}_{ch}", tag="mm")
        for ks in range(NDT):
            nc.tensor.matmul(pt, wbc_sb[ks],
                             xc[ks][:, ch * 512:(ch + 1) * 512],
                             start=(ks == 0), stop=(ks == NDT - 1))
        bb = yp.tile([N_STATE, 512], BF16, name=f"bb_{di}_{ch}", tag="bb", bufs=2)
        cc = yp.tile([N_STATE, 512], BF16, name=f"cc_{di}_{ch}", tag="cc", bufs=2)
        nc.scalar.copy(bb, pt[0:N_STATE, :])
        nc.scalar.copy(cc, pt[32:32 + N_STATE, :])
        nc.sync.dma_start(bstage[:, ch * 512:(ch + 1) * 512], bb)
        nc.sync.dma_start(cstage[:, ch * 512:(ch + 1) * 512], cc)

    # delta = softplus(W3 @ xc + dt_b) as ln(1+exp)
    delta = [big.tile([128, L], BF16, name=f"de_{di}_{dt}", tag=f"de{dt}",
                      bufs=2)
             for dt in range(NDT)]
    bdt_sb = const.tile([1, D_INNER], BF16, name=f"bdt_{di}")
    nc.sync.dma_start(bdt_sb, bdt_d[di, :, :])
    for mt in range(NDT):
        w3_mt = []
        for ks in range(NDT):
            w = wpool.tile([128, 128], BF16, name=f"w3_{di}_{mt}_{ks}", tag="wk",
                           bufs=16)
            nc.sync.dma_start(w, w3_d[di, ks, :, mt * 128:(mt + 1) * 128])
            w3_mt.append(w)
        for ch in range(NCH):
            pt = psum.tile([128, 512], F32, name=f"pde_{di}_{mt}_{ch}", tag="mm")
            for ks in range(NDT):
                nc.tensor.matmul(pt, w3_mt[ks],
                                 xc[ks][:, ch * 512:(ch + 1) * 512],
                                 start=(ks == 0), stop=False)
            nc.tensor.matmul(
                pt, bdt_sb[:, mt * 128:(mt + 1) * 128], ones_bf[:, 0:512],
                start=False, stop=True)
            dst = delta[mt][:, ch * 512:(ch + 1) * 512]
            tmp = yp.tile([128, 512], F32, name=f"sp_{di}_{mt}_{ch}", tag="sp",
                          bufs=2)
            nc.scalar.activation(tmp, pt, AF.Exp)
            nc.scalar.activation(dst, tmp, AF.Ln, bias=1.0)

    # z = silu(Wz @ x)
    wz_sb = []
    for ks in range(NET):
        w = wpool.tile([128, D_INNER], BF16, name=f"wz_{di}_{ks}", tag="wz")
        nc.sync.dma_start(w, wz_d[di, ks, :, :])
        wz_sb.append(w)
    zs_all = []
    for dt in range(NDT):
        zst = big.tile([128, L], BF16, name=f"zs_{di}_{dt}", tag=f"zs{dt}")
        for ch in range(NCH):
            zp = psum.tile([128, 512], F32, name=f"pz_{di}_{dt}_{ch}", tag="mm")
            for ks in range(NET):
                rhs = x_sb[ks][:, 3 + ch * 512: 3 + ch * 512 + 512]
                nc.tensor.matmul(zp, wz_sb[ks][:, dt * 128:(dt + 1) * 128],
                                 rhs, start=(ks == 0), stop=(ks == NET - 1))
            _silu(nc, yp, zst[:, ch * 512:(ch + 1) * 512], zp)
        zs_all.append(zst)

    return dict(a_scal=a_scal, x_sb=x_sb, ddiag=ddiag, xc=xc, delta=delta,
                bstage=bstage, cstage=cstage, u={}, zs_all=zs_all, y2={})


def _phase_scan(nc, di, st, pools, tens, dts):
    scanp, big, ypsum = pools["scanp"], pools["big"], pools["ypsum"]
    ident_bf = tens["ident_bf"]
    a_scal = st["a_scal"]
    for dt in dts:
        if dt not in st["u"]:
            ut = big.tile([128, L], BF16, name=f"u_{di}_{dt}", tag=f"u{dt}")
            nc.vector.tensor_tensor(ut, st["delta"][dt], st["xc"][dt], MUL)
            st["u"][dt] = ut
        yps = [ypsum.tile([128, 512], F32, name=f"yps_{di}_{dt}_{c}", tag="y")
               for c in range(NCH)]
        for n in range(N_STATE):
            e1 = (nc.sync, nc.scalar, nc.gpsimd)[n % 3]
            e2 = (nc.scalar, nc.gpsimd, nc.sync)[n % 3]
            bbc = scanp.tile([128, L], BF16, name=f"bbc_{di}_{dt}_{n}",
                             tag="bbc", bufs=4)
            srcb = st["bstage"][n:n + 1, :]
            e1.dma_start(
                bbc, bass.AP(tensor=srcb.tensor, offset=srcb.offset,
                             ap=[[0, 128]] + list(srcb.ap[1:])))
            cbc = scanp.tile([128, L], BF16, name=f"cbc_{di}_{dt}_{n}",
                             tag="cbc", bufs=4)
            srcc = st["cstage"][n:n + 1, :]
            e2.dma_start(
                cbc, bass.AP(tensor=srcc.tensor, offset=srcc.offset,
                             ap=[[0, 128]] + list(srcc.ap[1:])))

            da = scanp.tile([128, L], BF16, name=f"da_{di}_{dt}_{n}", tag="da")
            nc.scalar.activation(da, st["delta"][dt], AF.Exp,
                                 scale=float(a_scal[n]))
            dbx = scanp.tile([128, L], BF16, name=f"dbx_{di}_{dt}_{n}",
                             tag="dbx")
            nc.vector.tensor_tensor(dbx, st["u"][dt], bbc, MUL)
            h = scanp.tile([128, L], BF16, name=f"h_{di}_{dt}_{n}", tag="h")
            nc.vector.tensor_tensor_scan(h, da, dbx, 0.0, MUL, ADD)
            hc = scanp.tile([128, L], BF16, name=f"hc_{di}_{dt}_{n}", tag="hc")
            nc.vector.tensor_tensor(hc, h, cbc, MUL)
            for ch in range(NCH):
                nc.tensor.matmul(
                    yps[ch], ident_bf, hc[:, ch * 512:(ch + 1) * 512],
                    start=(n == 0), stop=False)

        # y2 = (y_scan + xc*D) * silu(z)
        y2t = big.tile([128, L], BF16, name=f"y2_{di}_{dt}", tag=f"de{dt}",
                       bufs=2)
        for ch in range(NCH):
            nc.tensor.matmul(yps[ch], st["ddiag"][dt],
                             st["xc"][dt][:, ch * 512:(ch + 1) * 512],
                             start=False, stop=True)
            nc.vector.tensor_tensor(
                y2t[:, ch * 512:(ch + 1) * 512], yps[ch],
                st["zs_all"][dt][:, ch * 512:(ch + 1) * 512], MUL)
        st["y2"][dt] = y2t


def _phase_tail(nc, di, st, pools, tens):
    wpool, yp, psum = pools["wpool"], pools["yp"], pools["psum"]
    wo_d, out_d = tens["wo_d"], tens["out_d"]
    for ot in range(NET):
        wo_sb = []
        for ks in range(NDT):
            w = wpool.tile([128, 128], BF16, name=f"wo_{di}_{ot}_{ks}",
                           tag="wk", bufs=16)
            nc.sync.dma_start(w, wo_d[di, ks, :, ot * 128:(ot + 1) * 128])
            wo_sb.append(w)
        for ch in range(NCH):
            pt = psum.tile([128, 512], F32, name=f"po_{di}_{ot}_{ch}", tag="mm")
            for ks in range(NDT):
                nc.tensor.matmul(pt, wo_sb[ks],
                                 st["y2"][ks][:, ch * 512:(ch + 1) * 512],
                                 start=(ks == 0), stop=(ks == NDT - 1))
            osb = yp.tile([128, 512], F32, name=f"os_{di}_{ot}_{ch}", tag="os",
                          bufs=2)
            nc.scalar.copy(osb, pt)
            nc.sync.dma_start(
                out_d[di, ot * 128:(ot + 1) * 128, ch * 512:(ch + 1) * 512],
                osb)


# ---------------------------------------------------------------------------
# host side
# ---------------------------------------------------------------------------

def _prep_dir(tw):
    in_w = tw["in_w"].astype(np.float64)        # (1024, 256)
    conv_w = tw["conv_w"].astype(np.float64)    # (512, 4)
    conv_b = tw["conv_b"].astype(np.float64)    # (512,)
    xproj = tw["xproj_w"].astype(np.float64)    # (48, 512)
    dt_w = tw["dt_w"].astype(np.float64)        # (512, 16)
    dt_b = tw["dt_b"].astype(np.float64)        # (512,)
    a_log = tw["A_log"].astype(np.float64)      # (512, 16)
    dvec = tw["D"].astype(np.float32)           # (512,)
    out_w = tw["out_w"].astype(np.float64)      # (256, 512)

    win_xi = in_w[:D_INNER]                     # (512, 256)
    win_z = in_w[D_INNER:]                      # (512, 256)

    w2 = np.zeros((8, 128, D_INNER), np.float64)
    for k in range(D_CONV):
        for t2 in range(NET):
            w2[k * NET + t2] = (win_xi[:, t2 * 128:(t2 + 1) * 128].T
                                * conv_w[:, k][None, :])
    bxc = conv_b[None, :]

    wz = np.stack([win_z[:, i * 128:(i + 1) * 128].T for i in range(NET)])

    w3_full = dt_w @ xproj[:DT_RANK]            # (512 di, 512 d)
    w3 = np.stack([w3_full.T[i * 128:(i + 1) * 128] for i in range(NDT)])
    bdt = dt_b[None, :]

    wbc_full = np.zeros((D_INNER, 64), np.float64)
    wbc_full[:, :N_STATE] = xproj[DT_RANK:DT_RANK + N_STATE].T
    wbc_full[:, 32:32 + N_STATE] = xproj[DT_RANK + N_STATE:].T
    wbc = np.stack([wbc_full[i * 128:(i + 1) * 128] for i in range(NDT)])
    wo = np.stack([out_w.T[i * 128:(i + 1) * 128] for i in range(NDT)])

    a_mat = -np.exp(a_log)
    assert np.allclose(a_mat, a_mat[0:1, :], rtol=1e-5, atol=1e-6), \
        "A_log rows differ across d; per-n scalar fast path invalid"
    ddiag = np.stack([np.diag(dvec[i * 128:(i + 1) * 128].astype(np.float64))
                      for i in range(NDT)])
    return dict(w2=w2, bxc=bxc, wz=wz, w3=w3, bdt=bdt, wbc=wbc, wo=wo,
                ddiag=ddiag, a_scal=a_mat[0])


def kernel(**inputs):
    x = np.asarray(inputs["x"], np.float32)     # (8, 256, 2048)

    prep = []
    for tag in ("fwd", "bwd"):
        tw = {k[len(tag) + 1:]: np.asarray(v) for k, v in inputs.items()
              if k.startswith(tag + "_")}
        prep.append(_prep_dir(tw))

    a_scal = [[float(v) for v in p["a_scal"]] for p in prep]
    key = ("nc", str(a_scal))
    if key not in _CACHE:
        _CACHE[key] = _build_nc(a_scal)
    nc = _CACHE[key]

    def st(arrs, dtype):
        return np.ascontiguousarray(
            np.stack([np.asarray(a) for a in arrs]).astype(dtype))

    common = dict(
        w2=st([p["w2"] for p in prep], bf),
        bxc=st([p["bxc"] for p in prep], bf),
        wz=st([p["wz"] for p in prep], bf),
        w3=st([p["w3"] for p in prep], bf),
        bdt=st([p["bdt"] for p in prep], bf),
        wbc=st([p["wbc"] for p in prep], bf),
        wo=st([p["wo"] for p in prep], bf),
        ddiag=st([p["ddiag"] for p in prep], bf),
        ident=np.eye(128, dtype=bf),
    )

    in_maps = []
    for b in range(B_SZ):
        xp = np.zeros((2, D_MODEL, L + 3), bf)
        xp[0, :, 3:] = x[b].astype(bf)
        xp[1, :, 3:] = x[b, :, ::-1].astype(bf)
        in_maps.append(dict(common, xp=xp))

    _LAST["in_maps"] = in_maps

    if SIM_COMPAT:
        from concourse.bass_interp import CoreSim
        nb = int(os.environ.get("BIMAMBA_SIM_NB", "1"))
        res = []
        for b_i in range(nb):
            sim = CoreSim(nc, trace=False)
            for k, v in in_maps[b_i].items():
                sim.tensor(k)[:] = v
            sim.simulate()
            res.append(dict(out=np.array(sim.tensor("out"))))
        while len(res) < B_SZ:
            res.append(res[-1])
    else:
        from concourse.bass_utils import run_bass_kernel_spmd
        r = run_bass_kernel_spmd(nc, in_maps, core_ids=list(range(B_SZ)))
        res = r.results

    out = np.empty((B_SZ, 2 * D_MODEL, L), np.float32)
    for b in range(B_SZ):
        o = res[b]["out"]
        out[b, :D_MODEL] = o[0]
        out[b, D_MODEL:] = o[1][:, ::-1]
    return out


# revision 16
# speedup vs baseline: 1.0977x; 1.0172x over previous
"""BiMamba Trainium2 Bass kernel.

Sharding: data-parallel over batch — 8 NeuronCores, one batch element each,
no collectives. Each core runs both directional Mamba blocks (fwd on x,
bwd on host-flipped x) in channel-major layout (d on partitions, L free).

Per direction (d_model=256, d_inner=512, n_state=16, dt_rank=16, d_conv=4,
L=2048):
  xc = silu(conv1d(W_in_xi @ x) + conv_b)   -- fused into one PE matmul via
       host-built W2[(k,d),e] = in_w[e,d]*conv_w[e,k] over shifted x views
  delta = softplus(W3 @ xc + dt_b),  W3 = dt_w @ xproj_dt  (host-fused)
  B,C   = xproj_bc @ xc               (staged to DRAM, DMA-broadcast per n)
  h_n[t] = exp(A_n*delta[t])*h_n[t-1] + delta[t]*xc[t]*B[n,t]  (DVE scan)
  y = sum_n C_n*h_n (Pool mul + PE identity-accumulate) + xc*D
  out = out_w @ (y * silu(z)),  z = W_z @ x

A_log is structurally log(arange(1..n_state+1)) broadcast over d (the
reference constructs it that way), so A_n is a per-n scalar — asserted at
runtime — allowing exp(A_n*delta) as one activation with a float scale.
"""

import os
from contextlib import ExitStack

import ml_dtypes
import numpy as np

import concourse.bacc as bacc
import concourse.bass as bass
import concourse.mybir as mybir
import concourse.tile as tile

F32 = mybir.dt.float32
BF16 = mybir.dt.bfloat16
AF = mybir.ActivationFunctionType
MUL = mybir.AluOpType.mult
ADD = mybir.AluOpType.add

D_MODEL = 256
N_STATE = 16
D_INNER = 512
DT_RANK = 16
D_CONV = 4
B_SZ, L = 8, 2048
NDT = D_INNER // 128          # 4 d-inner partition tiles
NCH = L // 512                # 4 free-dim chunks of 512
NET = D_MODEL // 128          # 2 d-model partition tiles

SIM_COMPAT = bool(int(os.environ.get("BIMAMBA_SIM", "0")))

bf = ml_dtypes.bfloat16

_CACHE = {}
_LAST = {}


def _build_nc(a_scal):
    """Build the single-core bass module (same NEFF for all 8 cores).
    a_scal: [2][16] python floats — compiled in as activation scales."""
    nc = bacc.Bacc("TRN2", target_bir_lowering=False, debug=False)

    xp_d = nc.dram_tensor("xp", [2, D_MODEL, L + 3], BF16, kind="ExternalInput")
    w2_d = nc.dram_tensor("w2", [2, 8, 128, D_INNER], BF16, kind="ExternalInput")
    bxc_d = nc.dram_tensor("bxc", [2, 1, D_INNER], BF16, kind="ExternalInput")
    wz_d = nc.dram_tensor("wz", [2, 2, 128, D_INNER], BF16, kind="ExternalInput")
    w3_d = nc.dram_tensor("w3", [2, 4, 128, D_INNER], BF16, kind="ExternalInput")
    bdt_d = nc.dram_tensor("bdt", [2, 1, D_INNER], BF16, kind="ExternalInput")
    wbc_d = nc.dram_tensor("wbc", [2, 4, 128, 64], BF16,
                           kind="ExternalInput")
    wo_d = nc.dram_tensor("wo", [2, 4, 128, D_MODEL], BF16, kind="ExternalInput")
    dd_d = nc.dram_tensor("ddiag", [2, 4, 128, 128], BF16, kind="ExternalInput")
    id_d = nc.dram_tensor("ident", [128, 128], BF16, kind="ExternalInput")
    out_d = nc.dram_tensor("out", [2, D_MODEL, L], F32, kind="ExternalOutput")

    with tile.TileContext(nc) as tc, ExitStack() as ctx:
        wpool = ctx.enter_context(tc.tile_pool(name="wpool", bufs=4))
        const = ctx.enter_context(tc.tile_pool(name="const", bufs=1))
        big = ctx.enter_context(tc.tile_pool(name="big", bufs=1))
        scanp = ctx.enter_context(tc.tile_pool(name="scanp", bufs=2))
        yp = ctx.enter_context(tc.tile_pool(name="yp", bufs=4))
        psum = ctx.enter_context(tc.tile_pool(name="psum", bufs=2, space="PSUM"))
        ypsum = ctx.enter_context(tc.tile_pool(name="ypsum", bufs=4, space="PSUM"))
        dram = ctx.enter_context(tc.tile_pool(name="dram", bufs=1, space="DRAM"))

        ones_bf = const.tile([1, 512], BF16)
        nc.vector.memset(ones_bf, 1.0)
        ident_bf = const.tile([128, 128], BF16)
        nc.sync.dma_start(ident_bf, id_d[:, :])

        pools = dict(wpool=wpool, const=const, big=big, scanp=scanp, yp=yp,
                     psum=psum, ypsum=ypsum, dram=dram)
        tens = dict(xp_d=xp_d, w2_d=w2_d, bxc_d=bxc_d, wz_d=wz_d, w3_d=w3_d,
                    bdt_d=bdt_d, wbc_d=wbc_d, wo_d=wo_d, dd_d=dd_d,
                    out_d=out_d, ones_bf=ones_bf, ident_bf=ident_bf)
        st0 = _phase_a1(nc, 0, a_scal[0], pools, tens)
        _phase_a2(nc, 0, st0, pools, tens)
        _phase_a3(nc, 0, st0, pools, tens)
        _phase_scan(nc, 0, st0, pools, tens, dts=[0])
        st1 = _phase_a1(nc, 1, a_scal[1], pools, tens)
        _phase_scan(nc, 0, st0, pools, tens, dts=[1])
        _phase_a2(nc, 1, st1, pools, tens)
        _phase_scan(nc, 0, st0, pools, tens, dts=[2])
        _phase_a3(nc, 1, st1, pools, tens)
        _phase_scan(nc, 0, st0, pools, tens, dts=[3])
        _phase_tail(nc, 0, st0, pools, tens)
        _phase_scan(nc, 1, st1, pools, tens, dts=[0, 1, 2, 3])
        _phase_tail(nc, 1, st1, pools, tens)

    nc.compile()
    return nc


def _silu(nc, yp, dst, src_psum):
    if SIM_COMPAT:
        sg = yp.tile(list(dst.shape), F32, name=f"sg_{nc.next_id()}", tag="sg",
                     bufs=2)
        nc.scalar.activation(sg, src_psum, AF.Sigmoid)
        nc.vector.tensor_tensor(dst, sg, src_psum, MUL)
    else:
        nc.scalar.activation(dst, src_psum, AF.Silu)


def _phase_a1(nc, di, a_scal, pools, tens):
    """Projections: x load, in-proj+conv->xc, delta, B/C staging, u, z."""
    wpool, const, big, yp = (pools[k] for k in ("wpool", "const", "big", "yp"))
    psum, dram = pools["psum"], pools["dram"]
    xp_d, w2_d, bxc_d = tens["xp_d"], tens["w2_d"], tens["bxc_d"]
    wz_d, w3_d, bdt_d, wbc_d = tens["wz_d"], tens["w3_d"], tens["bdt_d"], tens["wbc_d"]
    dd_d, ones_bf = tens["dd_d"], tens["ones_bf"]

    x_sb = []
    for t2 in range(NET):
        t = big.tile([128, L + 3], BF16, name=f"x_{di}_{t2}", tag=f"x{t2}", bufs=2)
        nc.sync.dma_start(t, xp_d[di, t2 * 128:(t2 + 1) * 128, :])
        x_sb.append(t)

    ddiag = []
    for dt in range(NDT):
        t = const.tile([128, 128], BF16, name=f"dd_{di}_{dt}")
        nc.sync.dma_start(t, dd_d[di, dt, :, :])
        ddiag.append(t)

    # in-proj + conv fused -> xc = silu(.)
    xc = [big.tile([128, L], BF16, name=f"xc_{di}_{dt}", tag=f"xc{dt}", bufs=2)
          for dt in range(NDT)]
    bxc_sb = const.tile([1, D_INNER], BF16, name=f"bxc_{di}")
    nc.sync.dma_start(bxc_sb, bxc_d[di, :, :])

    for et in range(NDT):
        w2_et = []
        for ks in range(8):
            w = wpool.tile([128, 128], BF16, name=f"w2_{di}_{et}_{ks}", tag="wk",
                           bufs=16)
            nc.sync.dma_start(w, w2_d[di, ks, :, et * 128:(et + 1) * 128])
            w2_et.append(w)
        for ch in range(NCH):
            pt = psum.tile([128, 512], F32, name=f"pxc_{di}_{et}_{ch}", tag="mm")
            for ks in range(8):
                k, t2 = ks // NET, ks % NET
                rhs = x_sb[t2][:, k + ch * 512: k + ch * 512 + 512]
                nc.tensor.matmul(pt, w2_et[ks], rhs, start=(ks == 0), stop=False)
            nc.tensor.matmul(
                pt, bxc_sb[:, et * 128:(et + 1) * 128], ones_bf[:, 0:512],
                start=False, stop=True)
            _silu(nc, yp, xc[et][:, ch * 512:(ch + 1) * 512], pt)

    return dict(a_scal=a_scal, x_sb=x_sb, ddiag=ddiag, xc=xc,
                u={}, y2={})


def _phase_a2(nc, di, st, pools, tens):
    wpool, const, big, yp = (pools[k] for k in ("wpool", "const", "big", "yp"))
    psum, dram = pools["psum"], pools["dram"]
    w3_d, bdt_d, wbc_d = tens["w3_d"], tens["bdt_d"], tens["wbc_d"]
    ones_bf = tens["ones_bf"]
    xc = st["xc"]
    # B,C rows -> DRAM staging (bf16)
    bstage = dram.tile([N_STATE, L], BF16, name=f"bst_{di}", tag=f"bst{di}")
    cstage = dram.tile([N_STATE, L], BF16, name=f"cst_{di}", tag=f"cst{di}")
    wbc_sb = []
    for ks in range(NDT):
        w = wpool.tile([128, 64], BF16, name=f"wbc_{di}_{ks}", tag="wbc")
        nc.sync.dma_start(w, wbc_d[di, ks, :, :])
        wbc_sb.append(w)
    for ch in range(NCH):
        pt = psum.tile([64, 512], F32, name=f"pbc_{di}_{ch}", tag="mm")
        for ks in range(NDT):
            nc.tensor.matmul(pt, wbc_sb[ks],
                             xc[ks][:, ch * 512:(ch + 1) * 512],
                             start=(ks == 0), stop=(ks == NDT - 1))
        bb = yp.tile([N_STATE, 512], BF16, name=f"bb_{di}_{ch}", tag="bb", bufs=2)
        cc = yp.tile([N_STATE, 512], BF16, name=f"cc_{di}_{ch}", tag="cc", bufs=2)
        nc.scalar.copy(bb, pt[0:N_STATE, :])
        nc.scalar.copy(cc, pt[32:32 + N_STATE, :])
        nc.sync.dma_start(bstage[:, ch * 512:(ch + 1) * 512], bb)
        nc.sync.dma_start(cstage[:, ch * 512:(ch + 1) * 512], cc)

    # delta = softplus(W3 @ xc + dt_b) as ln(1+exp)
    delta = [big.tile([128, L], BF16, name=f"de_{di}_{dt}", tag=f"de{dt}",
                      bufs=2)
             for dt in range(NDT)]
    bdt_sb = const.tile([1, D_INNER], BF16, name=f"bdt_{di}")
    nc.sync.dma_start(bdt_sb, bdt_d[di, :, :])
    for mt in range(NDT):
        w3_mt = []
        for ks in range(NDT):
            w = wpool.tile([128, 128], BF16, name=f"w3_{di}_{mt}_{ks}", tag="wk",
                           bufs=16)
            nc.sync.dma_start(w, w3_d[di, ks, :, mt * 128:(mt + 1) * 128])
            w3_mt.append(w)
        for ch in range(NCH):
            pt = psum.tile([128, 512], F32, name=f"pde_{di}_{mt}_{ch}", tag="mm")
            for ks in range(NDT):
                nc.tensor.matmul(pt, w3_mt[ks],
                                 xc[ks][:, ch * 512:(ch + 1) * 512],
                                 start=(ks == 0), stop=False)
            nc.tensor.matmul(
                pt, bdt_sb[:, mt * 128:(mt + 1) * 128], ones_bf[:, 0:512],
                start=False, stop=True)
            dst = delta[mt][:, ch * 512:(ch + 1) * 512]
            tmp = yp.tile([128, 512], F32, name=f"sp_{di}_{mt}_{ch}", tag="sp",
                          bufs=2)
            nc.scalar.activation(tmp, pt, AF.Exp)
            nc.scalar.activation(dst, tmp, AF.Ln, bias=1.0)

    st["bstage"], st["cstage"], st["delta"] = bstage, cstage, delta


def _phase_a3(nc, di, st, pools, tens):
    wpool, big, yp, psum = (pools[k] for k in ("wpool", "big", "yp", "psum"))
    wz_d = tens["wz_d"]
    x_sb = st["x_sb"]
    # z = silu(Wz @ x)
    wz_sb = []
    for ks in range(NET):
        w = wpool.tile([128, D_INNER], BF16, name=f"wz_{di}_{ks}", tag="wz")
        nc.sync.dma_start(w, wz_d[di, ks, :, :])
        wz_sb.append(w)
    zs_all = []
    for dt in range(NDT):
        zst = big.tile([128, L], BF16, name=f"zs_{di}_{dt}", tag=f"zs{dt}")
        for ch in range(NCH):
            zp = psum.tile([128, 512], F32, name=f"pz_{di}_{dt}_{ch}", tag="mm")
            for ks in range(NET):
                rhs = x_sb[ks][:, 3 + ch * 512: 3 + ch * 512 + 512]
                nc.tensor.matmul(zp, wz_sb[ks][:, dt * 128:(dt + 1) * 128],
                                 rhs, start=(ks == 0), stop=(ks == NET - 1))
            _silu(nc, yp, zst[:, ch * 512:(ch + 1) * 512], zp)
        zs_all.append(zst)

    st["zs_all"] = zs_all


def _phase_scan(nc, di, st, pools, tens, dts):
    scanp, big, ypsum = pools["scanp"], pools["big"], pools["ypsum"]
    ident_bf = tens["ident_bf"]
    a_scal = st["a_scal"]
    for dt in dts:
        if dt not in st["u"]:
            ut = big.tile([128, L], BF16, name=f"u_{di}_{dt}", tag=f"u{dt}")
            nc.vector.tensor_tensor(ut, st["delta"][dt], st["xc"][dt], MUL)
            st["u"][dt] = ut
        yps = [ypsum.tile([128, 512], F32, name=f"yps_{di}_{dt}_{c}", tag="y")
               for c in range(NCH)]
        for n in range(N_STATE):
            e1 = (nc.sync, nc.scalar, nc.gpsimd)[n % 3]
            e2 = (nc.scalar, nc.gpsimd, nc.sync)[n % 3]
            bbc = scanp.tile([128, L], BF16, name=f"bbc_{di}_{dt}_{n}",
                             tag="bbc", bufs=4)
            srcb = st["bstage"][n:n + 1, :]
            e1.dma_start(
                bbc, bass.AP(tensor=srcb.tensor, offset=srcb.offset,
                             ap=[[0, 128]] + list(srcb.ap[1:])))
            cbc = scanp.tile([128, L], BF16, name=f"cbc_{di}_{dt}_{n}",
                             tag="cbc", bufs=4)
            srcc = st["cstage"][n:n + 1, :]
            e2.dma_start(
                cbc, bass.AP(tensor=srcc.tensor, offset=srcc.offset,
                             ap=[[0, 128]] + list(srcc.ap[1:])))

            da = scanp.tile([128, L], BF16, name=f"da_{di}_{dt}_{n}", tag="da")
            nc.scalar.activation(da, st["delta"][dt], AF.Exp,
                                 scale=float(a_scal[n]))
            dbx = scanp.tile([128, L], BF16, name=f"dbx_{di}_{dt}_{n}",
                             tag="dbx")
            nc.vector.tensor_tensor(dbx, st["u"][dt], bbc, MUL)
            h = scanp.tile([128, L], BF16, name=f"h_{di}_{dt}_{n}", tag="h")
            nc.vector.tensor_tensor_scan(h, da, dbx, 0.0, MUL, ADD)
            hc = scanp.tile([128, L], BF16, name=f"hc_{di}_{dt}_{n}", tag="hc")
            nc.vector.tensor_tensor(hc, h, cbc, MUL)
            for ch in range(NCH):
                nc.tensor.matmul(
                    yps[ch], ident_bf, hc[:, ch * 512:(ch + 1) * 512],
                    start=(n == 0), stop=False)

        # y2 = (y_scan + xc*D) * silu(z)
        y2t = big.tile([128, L], BF16, name=f"y2_{di}_{dt}", tag=f"de{dt}",
                       bufs=2)
        for ch in range(NCH):
            nc.tensor.matmul(yps[ch], st["ddiag"][dt],
                             st["xc"][dt][:, ch * 512:(ch + 1) * 512],
                             start=False, stop=True)
            nc.vector.tensor_tensor(
                y2t[:, ch * 512:(ch + 1) * 512], yps[ch],
                st["zs_all"][dt][:, ch * 512:(ch + 1) * 512], MUL)
        st["y2"][dt] = y2t


def _phase_tail(nc, di, st, pools, tens):
    wpool, yp, psum = pools["wpool"], pools["yp"], pools["psum"]
    wo_d, out_d = tens["wo_d"], tens["out_d"]
    for ot in range(NET):
        wo_sb = []
        for ks in range(NDT):
            w = wpool.tile([128, 128], BF16, name=f"wo_{di}_{ot}_{ks}",
                           tag="wk", bufs=16)
            nc.sync.dma_start(w, wo_d[di, ks, :, ot * 128:(ot + 1) * 128])
            wo_sb.append(w)
        for ch in range(NCH):
            pt = psum.tile([128, 512], F32, name=f"po_{di}_{ot}_{ch}", tag="mm")
            for ks in range(NDT):
                nc.tensor.matmul(pt, wo_sb[ks],
                                 st["y2"][ks][:, ch * 512:(ch + 1) * 512],
                                 start=(ks == 0), stop=(ks == NDT - 1))
            osb = yp.tile([128, 512], F32, name=f"os_{di}_{ot}_{ch}", tag="os",
                          bufs=2)
            nc.scalar.copy(osb, pt)
            nc.sync.dma_start(
                out_d[di, ot * 128:(ot + 1) * 128, ch * 512:(ch + 1) * 512],
                osb)


# ---------------------------------------------------------------------------
# host side
# ---------------------------------------------------------------------------

def _prep_dir(tw):
    in_w = tw["in_w"].astype(np.float64)        # (1024, 256)
    conv_w = tw["conv_w"].astype(np.float64)    # (512, 4)
    conv_b = tw["conv_b"].astype(np.float64)    # (512,)
    xproj = tw["xproj_w"].astype(np.float64)    # (48, 512)
    dt_w = tw["dt_w"].astype(np.float64)        # (512, 16)
    dt_b = tw["dt_b"].astype(np.float64)        # (512,)
    a_log = tw["A_log"].astype(np.float64)      # (512, 16)
    dvec = tw["D"].astype(np.float32)           # (512,)
    out_w = tw["out_w"].astype(np.float64)      # (256, 512)

    win_xi = in_w[:D_INNER]                     # (512, 256)
    win_z = in_w[D_INNER:]                      # (512, 256)

    w2 = np.zeros((8, 128, D_INNER), np.float64)
    for k in range(D_CONV):
        for t2 in range(NET):
            w2[k * NET + t2] = (win_xi[:, t2 * 128:(t2 + 1) * 128].T
                                * conv_w[:, k][None, :])
    bxc = conv_b[None, :]

    wz = np.stack([win_z[:, i * 128:(i + 1) * 128].T for i in range(NET)])

    w3_full = dt_w @ xproj[:DT_RANK]            # (512 di, 512 d)
    w3 = np.stack([w3_full.T[i * 128:(i + 1) * 128] for i in range(NDT)])
    bdt = dt_b[None, :]

    wbc_full = np.zeros((D_INNER, 64), np.float64)
    wbc_full[:, :N_STATE] = xproj[DT_RANK:DT_RANK + N_STATE].T
    wbc_full[:, 32:32 + N_STATE] = xproj[DT_RANK + N_STATE:].T
    wbc = np.stack([wbc_full[i * 128:(i + 1) * 128] for i in range(NDT)])
    wo = np.stack([out_w.T[i * 128:(i + 1) * 128] for i in range(NDT)])

    a_mat = -np.exp(a_log)
    assert np.allclose(a_mat, a_mat[0:1, :], rtol=1e-5, atol=1e-6), \
        "A_log rows differ across d; per-n scalar fast path invalid"
    ddiag = np.stack([np.diag(dvec[i * 128:(i + 1) * 128].astype(np.float64))
                      for i in range(NDT)])
    return dict(w2=w2, bxc=bxc, wz=wz, w3=w3, bdt=bdt, wbc=wbc, wo=wo,
                ddiag=ddiag, a_scal=a_mat[0])


def kernel(**inputs):
    x = np.asarray(inputs["x"], np.float32)     # (8, 256, 2048)

    prep = []
    for tag in ("fwd", "bwd"):
        tw = {k[len(tag) + 1:]: np.asarray(v) for k, v in inputs.items()
              if k.startswith(tag + "_")}
        prep.append(_prep_dir(tw))

    a_scal = [[float(v) for v in p["a_scal"]] for p in prep]
    key = ("nc", str(a_scal))
    if key not in _CACHE:
        _CACHE[key] = _build_nc(a_scal)
    nc = _CACHE[key]

    def st(arrs, dtype):
        return np.ascontiguousarray(
            np.stack([np.asarray(a) for a in arrs]).astype(dtype))

    common = dict(
        w2=st([p["w2"] for p in prep], bf),
        bxc=st([p["bxc"] for p in prep], bf),
        wz=st([p["wz"] for p in prep], bf),
        w3=st([p["w3"] for p in prep], bf),
        bdt=st([p["bdt"] for p in prep], bf),
        wbc=st([p["wbc"] for p in prep], bf),
        wo=st([p["wo"] for p in prep], bf),
        ddiag=st([p["ddiag"] for p in prep], bf),
        ident=np.eye(128, dtype=bf),
    )

    in_maps = []
    for b in range(B_SZ):
        xp = np.zeros((2, D_MODEL, L + 3), bf)
        xp[0, :, 3:] = x[b].astype(bf)
        xp[1, :, 3:] = x[b, :, ::-1].astype(bf)
        in_maps.append(dict(common, xp=xp))

    _LAST["in_maps"] = in_maps

    if SIM_COMPAT:
        from concourse.bass_interp import CoreSim
        nb = int(os.environ.get("BIMAMBA_SIM_NB", "1"))
        res = []
        for b_i in range(nb):
            sim = CoreSim(nc, trace=False)
            for k, v in in_maps[b_i].items():
                sim.tensor(k)[:] = v
            sim.simulate()
            res.append(dict(out=np.array(sim.tensor("out"))))
        while len(res) < B_SZ:
            res.append(res[-1])
    else:
        from concourse.bass_utils import run_bass_kernel_spmd
        r = run_bass_kernel_spmd(nc, in_maps, core_ids=list(range(B_SZ)))
        res = r.results

    out = np.empty((B_SZ, 2 * D_MODEL, L), np.float32)
    for b in range(B_SZ):
        o = res[b]["out"]
        out[b, :D_MODEL] = o[0]
        out[b, D_MODEL:] = o[1][:, ::-1]
    return out
